# revision 19
# baseline (speedup 1.0000x reference)
"""MaxSim ranker kernel for 8 Trainium2 NeuronCores.

Sharding (per the hint): the vectors table is split across the 8 cores (each
shard = its own doc set), q_vectors replicated, one SPMD program; each core
scores its docs and the host merges the per-core results into the global
top-k, exactly like the repo's multi-shard MaxSim.

Numerics: exact 3-pass bf16x2 matmul (hi/lo split of q and V:
qh*vh + qh*vl + ql*vh ~ fp32 precision), fp32 PSUM accumulate, fp32 max /
sum / sort. This preserves the fp32 reference's top-k ordering bit-for-bit
(rel err ~1e-6, 0/800 pid mismatches vs a float64 oracle).

Performance structure (~142 us/core vs 341 us for the naive full-table
version):
  - Candidate packing: each batch row only touches the ~926 docs its tokens
    hit, so only ~500/625 docs per core are needed by ANY row and only ~350
    by each 4-row half of the 256-row replicated q block (qc0 = batches 0-3,
    qc1 = 4-7). The host packs V columns by category [only-qc0 | shared |
    only-qc1] (8-doc groups), each qc's matmuls cover only its groups:
    ~1.8x less PE work, ~1.15x less HBM than scoring everything.
  - Doc->core assignment is rebalanced (categories dealt round-robin) so all
    cores get identical span sizes (minimal padding).
  - Stream order interleaves categories (largest-remainder) so PE load per
    chunk is uniform; a few shared groups are front-loaded to cover the DMA
    cold-start. Small first chunks let PE start at ~10 us.
  - V streams are chunk-major (one DRAM tensor per chunk per stream) so each
    DMA moves [128, W*1024] bf16 with W*2KB contiguous per partition line;
    4 slices/chunk round-robin over the sync/scalar/gpsimd queues sustain
    ~300+ GB/s.
  - pid gather + candidate sets are computed on host (emb2pid[token_ids] is
    an 8K-element lookup); no indirect DMA, no on-device mask - the host
    filters (row, doc) validity at merge time.
  - Two consecutive same-qc groups share one [128, 2048] PSUM tile and a
    single 2048-wide DVE tensor_reduce (DVE is the only engine that can
    max-reduce from PSUM; this halves its per-instruction access bubble).
  - q-sums (block-ones fp32 matmul over the 32 queries of each batch) are
    emitted incrementally per slot block, ACT copies PSUM->SBUF, so the
    post-loop tail is only the last block.
"""

import sys

for _p in ("/opt/trn_rl_repo", "/root/.axon_site/_ro/trn_rl_repo"):
    if _p not in sys.path:
        sys.path.append(_p)

import numpy as np
import ml_dtypes

# ---- problem constants (hardcoded per contract) ----
N_DOCS = 5000
DOC_LEN = 128
DIM = 128
B = 8
NQ = 32
NTOK = 1024
N_EMB = N_DOCS * DOC_LEN
NCORES = 8
SHARD = N_DOCS // NCORES           # 625 docs per core
GROUP = 1024                       # matmul/reduce group: 8 docs
GDOCS = GROUP // DOC_LEN           # 8 docs per group
NEG = -1.0e30

_PROGRAMS = {}


def _chunk_plan(g_tot):
    """Chunk sizes in groups: small first chunks so PE starts early."""
    chunks = []
    for w in (1, 2, 4):
        if sum(chunks) + w <= g_tot:
            chunks.append(w)
    while g_tot - sum(chunks) >= 5:
        chunks.append(5)
    if g_tot - sum(chunks):
        chunks.append(g_tot - sum(chunks))
    return chunks


def _span_pattern(g_o0, g_sh, g_o1, front_sh=6):
    """Interleave group categories so PE load (1 qc for only-spans, 2 for
    shared) is uniform across the stream. The first few groups are all
    'shared' (PE-heaviest per byte) to cover the DMA cold-start ramp."""
    front = min(front_sh, g_sh)
    counts = {"o0": g_o0, "sh": g_sh - front, "o1": g_o1}
    tot = sum(counts.values())
    placed = {k: 0 for k in counts}
    pat = ["sh"] * front
    for i in range(tot):
        # pick the category furthest behind its proportional share
        best, bscore = None, None
        for k in ("sh", "o0", "o1"):
            if placed[k] >= counts[k]:
                continue
            score = (placed[k] + 1) / counts[k]
            if bscore is None or score < bscore:
                best, bscore = k, score
        pat.append(best)
        placed[best] += 1
    return pat


def _build_program(g_o0, g_sh, g_o1, v_bufs=6, psum_bufs=4):
    """Packed-span program. Groups laid out [only0 | shared | only1].

    qc0 (q rows 0-127 = batches 0-3) covers groups [0, g_o0+g_sh);
    qc1 (q rows 128-255 = batches 4-7) covers groups [g_o0, g_tot).
    V streams are chunk-major: one DRAM tensor per chunk per stream so each
    DMA moves [128, W*1024] with W*2KB contiguous per partition line.
    """
    import concourse.bass as bass
    import concourse.mybir as mybir
    import concourse.tile as tile
    from concourse import bacc

    bf16 = mybir.dt.bfloat16
    f32 = mybir.dt.float32

    g_tot = g_o0 + g_sh + g_o1
    ga = g_o0 + g_sh                  # qc0 groups
    gb = g_sh + g_o1                  # qc1 groups
    a_docs = ga * GDOCS
    b_docs = gb * GDOCS

    nc = bacc.Bacc("TRN2", target_bir_lowering=False, debug=False)

    chunks = _chunk_plan(g_tot)
    qw = nc.dram_tensor("qw", [DIM, 512], bf16, kind="ExternalInput")
    vh_c = [nc.dram_tensor(f"vh{i}", [DIM, w * GROUP], bf16, kind="ExternalInput")
            for i, w in enumerate(chunks)]
    vl_c = [nc.dram_tensor(f"vl{i}", [DIM, w * GROUP], bf16, kind="ExternalInput")
            for i, w in enumerate(chunks)]
    scr0 = nc.dram_tensor("scr0", [4, a_docs], f32, kind="ExternalOutput")
    scr1 = nc.dram_tensor("scr1", [4, b_docs], f32, kind="ExternalOutput")

    with tile.TileContext(nc) as tc:
        with (
            tc.tile_pool(name="const", bufs=1) as cpool,
            tc.tile_pool(name="v", bufs=v_bufs) as vpool,
            tc.tile_pool(name="st", bufs=4) as spool,
            tc.tile_pool(name="ps", bufs=psum_bufs, space="PSUM") as pspool,
            tc.tile_pool(name="res", bufs=1) as rpool,
        ):
            qw_sb = cpool.tile([DIM, 512], bf16)
            nc.gpsimd.dma_start(qw_sb[:], qw[:])

            # PE p-state warm-up: the Tensor queue only becomes ready ~8us in
            # (engine boot) and the first chunk lands ~10.5us; exactly 4 dummy
            # matmuls (~634ns each, mid-clock) fill that window so the real
            # stream starts closer to full clock without ever delaying it.
            warm = cpool.tile([DIM, 512], bf16)
            nc.vector.memset(warm[:], 0.0)
            wps = pspool.tile([128, 512], f32, tag="ps")
            for _ in range(4):
                nc.tensor.matmul(wps[:], warm[:, :128], warm[:],
                                 start=True, stop=True)

            # ones weights: col j sums queries of batch-block row j
            ones0 = cpool.tile([DIM, 4], f32)
            ones1 = cpool.tile([DIM, 4], f32)
            nc.vector.memset(ones0[:], 0.0)
            nc.vector.memset(ones1[:], 0.0)
            for j in range(4):
                nc.vector.memset(ones0[32 * j : 32 * j + 32, j : j + 1], 1.0)
                nc.vector.memset(ones1[32 * j : 32 * j + 32, j : j + 1], 1.0)

            maxres0 = rpool.tile([128, a_docs], f32)
            maxres1 = rpool.tile([128, b_docs], f32)

            CHMAX = GROUP * max(chunks)
            pattern = _span_pattern(g_o0, g_sh, g_o1)
            queues = [nc.sync, nc.scalar, nc.gpsimd]
            qi = 0
            slot = [0, 0]             # next output slot per qc
            g0 = 0

            # incremental q-sum: emit per-block ones-matmul + writeback a few
            # qc-units after the block's last reduce, so only the final block
            # sits in the post-loop tail.
            out_sb0 = rpool.tile([4, a_docs], f32)
            out_sb1 = rpool.tile([4, b_docs], f32)
            out_sb = (out_sb0, out_sb1)
            onesw = (ones0, ones1)
            scrs = (scr0, scr1)
            nblk = 3
            bounds = []
            for qcn, gq in ((0, ga), (1, gb)):
                bw = -(-gq // nblk)
                bounds.append(list(range(bw, gq, bw)) + [gq])
            nextb = [0, 0]
            pending = []              # (qc, lo_slot, hi_slot, ready_tseq)
            tseq = 0

            def emit_qsum(qc, lo_s, hi_s):
                lo, hi = lo_s * GDOCS, hi_s * GDOCS
                mres = maxres0 if qc == 0 else maxres1
                ps_s = pspool.tile([4, hi - lo], f32, tag="ps")
                nc.tensor.matmul(ps_s[:], onesw[qc][:], mres[:, lo:hi],
                                 start=True, stop=True)
                nc.vector.tensor_copy(out=out_sb[qc][:, lo:hi], in_=ps_s[:])
                nc.sync.dma_start(scrs[qc][:, lo:hi], out_sb[qc][:, lo:hi])

            for ci, w in enumerate(chunks):
                CH = w * GROUP
                vh_t = vpool.tile([DIM, CHMAX], bf16, tag="vh")
                vl_t = vpool.tile([DIM, CHMAX], bf16, tag="vl")
                # 4 DMA slices per chunk round-robin over the 3 queues
                hh = CH // 2
                for (dst, src) in ((vh_t[:, :hh], vh_c[ci][:, :hh]),
                                   (vh_t[:, hh:CH], vh_c[ci][:, hh:]),
                                   (vl_t[:, :hh], vl_c[ci][:, :hh]),
                                   (vl_t[:, hh:CH], vl_c[ci][:, hh:])):
                    queues[qi % 3].dma_start(dst, src)
                    qi += 1
                for gi in range(w):
                    g = g0 + gi
                    cat = pattern[g]
                    qcs = (0,) if cat == "o0" else (1,) if cat == "o1" else (0, 1)
                    for p in [p for p in pending if p[3] <= tseq]:
                        pending.remove(p)
                        emit_qsum(p[0], p[1], p[2])
                    for qc in qcs:
                        ps = pspool.tile([128, GROUP], f32, tag="ps")
                        qh = qw_sb[:, 128 * qc : 128 * qc + 128]
                        ql = qw_sb[:, 256 + 128 * qc : 256 + 128 * qc + 128]
                        for s in range(GROUP // 512):
                            sl = slice(512 * s, 512 * (s + 1))
                            gsl = slice(gi * GROUP + 512 * s, gi * GROUP + 512 * (s + 1))
                            nc.tensor.matmul(ps[:, sl], qh, vh_t[:, gsl], start=True, stop=False)
                            nc.tensor.matmul(ps[:, sl], qh, vl_t[:, gsl], start=False, stop=False)
                            nc.tensor.matmul(ps[:, sl], ql, vh_t[:, gsl], start=False, stop=True)
                        # output slot range for this (qc, g)
                        mres = maxres0 if qc == 0 else maxres1
                        base = slot[qc] * GDOCS
                        slot[qc] += 1
                        osl = slice(base, base + GDOCS)
                        # DVE is the only engine that can max-reduce (ACT has
                        # no max, Pool has no PSUM port / elementwise opcode).
                        nc.vector.tensor_reduce(
                            out=mres[:, osl],
                            in_=ps[:].rearrange("p (d t) -> p d t", t=DOC_LEN),
                            axis=mybir.AxisListType.X,
                            op=mybir.AluOpType.max,
                        )
                        tseq += 1
                        if nextb[qc] < len(bounds[qc]) and \
                                slot[qc] == bounds[qc][nextb[qc]]:
                            lo_s = bounds[qc][nextb[qc] - 1] if nextb[qc] else 0
                            pending.append((qc, lo_s, slot[qc], tseq + 5))
                            nextb[qc] += 1
                g0 += w
            assert slot[0] == ga and slot[1] == gb, (slot, ga, gb)
            for (qc, lo_s, hi_s, _) in pending:
                emit_qsum(qc, lo_s, hi_s)

    nc.compile()
    return nc


def _get_program(key_sizes, **kw):
    key = (key_sizes, tuple(sorted(kw.items())))
    if key not in _PROGRAMS:
        _PROGRAMS[key] = _build_program(*key_sizes, **kw)
    return _PROGRAMS[key]


def _bf16_split(x):
    hi = x.astype(ml_dtypes.bfloat16)
    lo = (x - hi.astype(np.float32)).astype(ml_dtypes.bfloat16)
    return hi, lo


def _plan(token_ids, emb2pid):
    """Candidate sets -> per-core packed doc lists + padded span sizes."""
    tok = np.asarray(token_ids).astype(np.int64)
    e2p = np.asarray(emb2pid).astype(np.int64)
    pids = e2p[tok]                                  # [B, NTOK]
    pids = np.where((pids < 0) | (pids >= N_DOCS), -1, pids)
    need = np.zeros((B, N_DOCS), bool)
    for b in range(B):
        p = pids[b]
        need[b, p[p >= 0]] = True
    n0 = need[:4].any(0)
    n1 = need[4:].any(0)
    # balanced assignment: deal each category's docs round-robin to cores
    # (the doc->shard mapping is ours to choose; merge uses explicit doc ids)
    cat0 = np.nonzero(n0 & ~n1)[0]
    cat1 = np.nonzero(n1 & ~n0)[0]
    cat2 = np.nonzero(n0 & n1)[0]
    docs = []   # per core: (o0_list, sh_list, o1_list)
    for c in range(NCORES):
        docs.append((cat0[c::NCORES], cat2[c::NCORES], cat1[c::NCORES]))

    def pad_g(n):
        return (int(n) + GDOCS - 1) // GDOCS

    g_o0 = pad_g(max(len(d[0]) for d in docs))
    g_sh = pad_g(max(len(d[1]) for d in docs))
    g_o1 = pad_g(max(len(d[2]) for d in docs))
    return need, docs, (g_o0, g_sh, g_o1)


def _prepare(q_vectors, vectors, docs, spans):
    g_o0, g_sh, g_o1 = spans
    g_tot = g_o0 + g_sh + g_o1
    cols = g_tot * GROUP
    n_slots = g_tot * GDOCS
    chunks = _chunk_plan(g_tot)
    pattern = _span_pattern(g_o0, g_sh, g_o1)

    q = np.ascontiguousarray(np.asarray(q_vectors, dtype=np.float32))
    V = np.asarray(vectors, dtype=np.float32)
    qt = np.ascontiguousarray(q.reshape(B * NQ, DIM).T)       # [128, 256]
    qh, ql = _bf16_split(qt)
    qw_np = np.concatenate([qh, ql], axis=1)                  # [128, 512]

    ga = g_o0 + g_sh
    gb = g_sh + g_o1
    in_maps = []
    slot_docs = np.full((NCORES, n_slots), -1, np.int64)      # stream order
    slot0 = np.full((NCORES, ga * GDOCS), -1, np.int64)       # qc0 outputs
    slot1 = np.full((NCORES, gb * GDOCS), -1, np.int64)       # qc1 outputs
    for c in range(NCORES):
        cat_docs = {"o0": list(docs[c][0]), "sh": list(docs[c][1]),
                    "o1": list(docs[c][2])}
        ptr = {"o0": 0, "sh": 0, "o1": 0}
        s0 = s1 = 0
        for g, cat in enumerate(pattern):
            lst = cat_docs[cat]
            p = ptr[cat]
            grp = lst[p : p + GDOCS]
            ptr[cat] = p + GDOCS
            grp = grp + [-1] * (GDOCS - len(grp))
            slot_docs[c, g * GDOCS : (g + 1) * GDOCS] = grp
            if cat in ("o0", "sh"):
                slot0[c, s0 * GDOCS : (s0 + 1) * GDOCS] = grp
                s0 += 1
            if cat in ("o1", "sh"):
                slot1[c, s1 * GDOCS : (s1 + 1) * GDOCS] = grp
                s1 += 1
        # gather V columns for this core's packed docs (stream order)
        vt = np.zeros((DIM, cols), np.float32)
        sel = slot_docs[c]
        valid = sel >= 0
        # V[doc] is [128 tok, 128 dim]; packed col block for slot j is V[doc].T
        vv = V[sel[valid]]                                    # [n, 128, 128]
        vt_v = vv.transpose(2, 0, 1).reshape(DIM, -1)         # [128, n*128]
        idx = np.nonzero(valid)[0]
        vt3 = vt.reshape(DIM, n_slots, DOC_LEN)
        vt3[:, idx, :] = vt_v.reshape(DIM, -1, DOC_LEN)
        vh_np, vl_np = _bf16_split(vt)
        m = {"qw": qw_np}
        g0 = 0
        for ci, w in enumerate(chunks):
            csl = slice(g0 * GROUP, (g0 + w) * GROUP)
            m[f"vh{ci}"] = np.ascontiguousarray(vh_np[:, csl])
            m[f"vl{ci}"] = np.ascontiguousarray(vl_np[:, csl])
            g0 += w
        in_maps.append(m)
    return in_maps, (slot0, slot1)


def _merge(results, need, slot_docs, spans, k_val):
    slot0, slot1 = slot_docs
    top_scores = np.empty((B, k_val), np.float32)
    top_pids = np.empty((B, k_val), np.int32)
    allv = [[] for _ in range(B)]
    allp = [[] for _ in range(B)]
    for c in range(NCORES):
        s0 = np.asarray(results[c]["scr0"], np.float32)       # [4, a_docs]
        s1 = np.asarray(results[c]["scr1"], np.float32)       # [4, b_docs]
        d0 = slot0[c]
        d1 = slot1[c]
        for j in range(4):
            for b, s, d in ((j, s0[j], d0), (4 + j, s1[j], d1)):
                m = (d >= 0) & need[b, np.maximum(d, 0)]
                allv[b].append(s[m])
                allp[b].append(d[m])
    for b in range(B):
        v = np.concatenate(allv[b])
        p = np.concatenate(allp[b])
        order = np.argsort(-v, kind="stable")[:k_val]
        top_scores[b] = v[order]
        top_pids[b] = p[order].astype(np.int32)
    return top_scores, top_pids


def _run(inputs, trace=False, trace_kwargs=None, build_kwargs=None):
    from concourse.bass_utils import run_bass_kernel_spmd

    need, docs, spans = _plan(inputs["token_ids"], inputs["emb2pid"])
    nc = _get_program(spans, **(build_kwargs or {}))
    in_maps, slot_docs = _prepare(inputs["q_vectors"], inputs["vectors"], docs, spans)
    br = run_bass_kernel_spmd(
        nc, in_maps, list(range(NCORES)), trace=trace, **(trace_kwargs or {})
    )
    k_val = int(np.asarray(inputs.get("k", 100)))
    outs = _merge(br.results, need, slot_docs, spans, k_val)
    return outs, br


def kernel(q_vectors, token_ids, vectors, emb2pid, k=100):
    outs, _ = _run(
        {
            "q_vectors": q_vectors,
            "token_ids": token_ids,
            "vectors": vectors,
            "emb2pid": emb2pid,
            "k": k,
        }
    )
    return outs

